# revision 17
# baseline (speedup 1.0000x reference)
"""Trainium2 Bass kernel for nn_FACoef.

Computes, for each batch b of x (B, 512, 512):
    out[b] = sum_{i<3, j<3} coef[i,j] * sum_elems((x_b^(i+2)) ** (j+1)) / (N*N)^(i+j+2)

Strategy (pure data parallel, 8 batches per core on 8 NeuronCores):
  Work with y = x^T.  y^k = (x^k)^T, and the elementwise power-sums are
  transpose invariant, so we compute the chain y2 = y@y, y3 = y@y2,
  y4 = y@y3 on the PE with the *natural-layout* x as the stationary
  operand (lhsT = (y)^T = x), needing only one PE transpose of x per
  batch to seed the chain's first rhs.

  Matmuls run in float32r (single-pass FP22 multiply, full PE rate).
  Per result matrix (128x2048 row-block-major layout):
    - ScalarE: Copy psum->sbuf with fused accum  -> s1 partials (+ rhs copy)
    - ScalarE: Square (first RA blocks) + accum  -> s2a partials, t2a
    - VectorE: square (rest) via scalar_tensor_tensor + accum -> s2b, t2b
    - VectorE: tensor_tensor_reduce t2*y + accum -> s3 partials
  Per-partition partial sums land in accumulator tiles, DMA'd out; the
  host reduces partitions and applies coef/norm in float64.
"""

import numpy as np

import concourse.bacc as bacc
import concourse.bass as bass
import concourse.mybir as mybir
import concourse.tile as tile
from concourse.bass_utils import run_bass_kernel_spmd

N = 512
RB = 4  # row blocks of 128
BPC = 8  # batches per core
NCORES = 8
ROWS = 3
COLS = 3
RA = 2  # r-blocks of the square pass done on ScalarE (rest on VectorE)

FP32 = mybir.dt.float32
FP32R = mybir.dt.float32r
AF = mybir.ActivationFunctionType
ALU = mybir.AluOpType


def build_nc():
    nc = bacc.Bacc(None, target_bir_lowering=False)
    x_ext = nc.declare_dram_parameter("x", [BPC, N, N], FP32, isOutput=False)
    ident_ext = nc.declare_dram_parameter("ident", [128, 128], FP32, isOutput=False)
    # acc_a: per (batch, mat): [s1, s2a];  acc_d: [s2b, s3a, s3b]
    acc_a_ext = nc.declare_dram_parameter("acc_a", [128, BPC * ROWS * 2], FP32, isOutput=True)
    acc_d_ext = nc.declare_dram_parameter("acc_d", [128, BPC * ROWS * 3], FP32, isOutput=True)

    with tile.TileContext(nc) as tc:
        with (
            tc.tile_pool(name="consts", bufs=1) as consts,
            tc.tile_pool(name="xpool", bufs=2) as xpool,
            tc.tile_pool(name="ypool", bufs=4) as ypool,
            tc.tile_pool(name="tpool", bufs=2) as tpool,
            tc.tile_pool(name="accpool", bufs=1) as accpool,
            tc.tile_pool(name="ps", bufs=2, space="PSUM") as pspool,
        ):
            identity = consts.tile([128, 128], FP32R)
            nc.sync.dma_start(out=identity, in_=ident_ext[:, :].bitcast(FP32R))
            # Dummy transpose: makes the PE observe the identity-producer
            # (gpsimd) semaphore once, so no real transpose/matmul has to
            # carry that wait — the fused-LDW slot only fits 2 sync waits.
            ps_warm = pspool.tile([128, RB * N], FP32, tag="ps")
            nc.tensor.transpose(
                ps_warm[:, 0:128].bitcast(FP32R), identity, identity
            )
            acc_a = accpool.tile([128, BPC * ROWS * 2], FP32)
            acc_d = accpool.tile([128, BPC * ROWS * 3], FP32)

            for b in range(BPC):
                # load x[b] in row-block-major layout: sbx[p, r, c] = x[128r+p, c]
                sbx = xpool.tile([128, RB, N], FP32R, tag="sbx")
                nc.sync.dma_start(
                    out=sbx,
                    in_=x_ext[b].rearrange("(r p) c -> p r c", p=128).bitcast(FP32R),
                )

                # y = x^T via 16 PE transposes: psT[q, r, 128j+p] = x[128j+p, 128r+q]
                psT = pspool.tile([128, RB * N], FP32, tag="ps")
                for r in range(RB):
                    for j in range(RB):
                        nc.tensor.transpose(
                            psT[:, r * N + 128 * j : r * N + 128 * (j + 1)].bitcast(
                                FP32R
                            ),
                            sbx[:, j, 128 * r : 128 * (r + 1)],
                            identity,
                        )
                ycur = ypool.tile([128, RB * N], FP32R, tag="y")
                nc.scalar.activation(ycur, psT, AF.Copy)

                for k in range(ROWS):  # y2, y3, y4
                    psY = pspool.tile([128, RB * N], FP32, tag="ps")
                    for m in range(RB):
                        for kk in range(RB):
                            nc.tensor.matmul(
                                psY[:, m * N : (m + 1) * N],
                                lhsT=sbx[:, kk, 128 * m : 128 * (m + 1)],
                                rhs=ycur[:, kk * N : (kk + 1) * N],
                                start=(kk == 0),
                                stop=(kk == RB - 1),
                            )
                    ysb = ypool.tile([128, RB * N], FP32R, tag="y")
                    ci = b * ROWS + k
                    # copy psum->sbuf + s1 partials
                    nc.scalar.activation(
                        ysb, psY, AF.Copy, accum_out=acc_a[:, 2 * ci + 1 : 2 * ci + 2]
                    )
                    # squares: ScalarE on first RA blocks, VectorE on the rest
                    t2a = tpool.tile([128, RA * N], FP32, tag="t2a")
                    nc.scalar.activation(
                        t2a,
                        ysb[:, : RA * N].bitcast(FP32),
                        AF.Square,
                        accum_out=acc_a[:, 2 * ci : 2 * ci + 1],
                    )
                    t2b = tpool.tile([128, (RB - RA) * N], FP32, tag="t2b")
                    nc.vector.scalar_tensor_tensor(
                        out=t2b,
                        in0=ysb[:, RA * N :].bitcast(FP32),
                        scalar=1.0,
                        in1=ysb[:, RA * N :].bitcast(FP32),
                        op0=ALU.mult,
                        op1=ALU.mult,
                        accum_out=acc_d[:, 3 * ci : 3 * ci + 1],
                    )
                    # cubes: t3 = t2 * y with fused reduction
                    t3a = tpool.tile([128, RA * N], FP32, tag="t3a")
                    nc.vector.affine_mul_reduce(
                        out=t3a,
                        accum_out=acc_d[:, 3 * ci + 1 : 3 * ci + 2],
                        in0=t2a,
                        in1=ysb[:, : RA * N].bitcast(FP32),
                        scale=1.0,
                        bias=0.0,
                    )
                    t3b = tpool.tile([128, (RB - RA) * N], FP32, tag="t3b")
                    nc.vector.affine_mul_reduce(
                        out=t3b,
                        accum_out=acc_d[:, 3 * ci + 2 : 3 * ci + 3],
                        in0=t2b,
                        in1=ysb[:, RA * N :].bitcast(FP32),
                        scale=1.0,
                        bias=0.0,
                    )
                    ycur = ysb

            nc.sync.dma_start(out=acc_a_ext[:, :], in_=acc_a)
            nc.sync.dma_start(out=acc_d_ext[:, :], in_=acc_d)

    # The fused-LDW fp32 matmul encoding only fits ONE sync wait.  Tile
    # emits conservative PE-self waits for PSUM slot reuse (WAW vs earlier
    # matmuls), but PE matmuls complete strictly in program order through
    # the single PSUM write port, so those waits are redundant — drop them.
    import os
    if os.environ.get("NO_STRIP_PE_WAITS", "") != "1":
        for bb in nc.m.functions[0].blocks:
            for ins in bb.instructions:
                if type(ins).__name__ == "InstMatmult":
                    si = ins.sync_info
                    own = {u.id for u in si.on_update}
                    kept = [w for w in si.on_wait if w.id not in own]
                    if len(kept) != len(si.on_wait):
                        si.on_wait = kept
                        ins.sync_info = si

    nc.finalize()
    return nc


_NC_CACHE = None


def get_nc():
    global _NC_CACHE
    if _NC_CACHE is None:
        _NC_CACHE = build_nc()
    return _NC_CACHE


def combine_partials(acc_a, acc_d, coef, out, base):
    """Reduce per-partition partials and apply coef/norm in float64."""
    a = acc_a.astype(np.float64).sum(axis=0)  # (BPC*ROWS*2,)
    d = acc_d.astype(np.float64).sum(axis=0)  # (BPC*ROWS*3,)
    norm_pow = (
        np.arange(COLS)[None, :] + np.arange(ROWS)[:, None] + 2
    ).astype(np.float64)
    w = coef.astype(np.float64) / (float(N * N) ** norm_pow)  # (ROWS, COLS)
    for b in range(BPC):
        acc = 0.0
        for i in range(ROWS):
            ci = b * ROWS + i
            s1 = a[2 * ci + 1]
            s2 = a[2 * ci] + d[3 * ci]
            s3 = d[3 * ci + 1] + d[3 * ci + 2]
            acc += w[i, 0] * s1 + w[i, 1] * s2 + w[i, 2] * s3
        out[base + b] = acc


def kernel(x, coef):
    x = np.ascontiguousarray(x, dtype=np.float32)
    coef = np.asarray(coef, dtype=np.float32)
    B = x.shape[0]
    assert B == BPC * NCORES and x.shape[1:] == (N, N)

    nc = get_nc()
    ident = np.eye(128, dtype=np.float32)
    in_maps = [
        {"x": x[c * BPC : (c + 1) * BPC], "ident": ident} for c in range(NCORES)
    ]
    res = run_bass_kernel_spmd(nc, in_maps, list(range(NCORES))).results

    out = np.zeros(B, dtype=np.float64)
    for c in range(NCORES):
        combine_partials(res[c]["acc_a"], res[c]["acc_d"], coef, out, c * BPC)
    return out.astype(np.float32)
